# revision 4
# baseline (speedup 1.0000x reference)
"""Trainium2 Bass kernel for nn_Decoder_80779744903970.

LSTM decoder: emb lookup + x_proj = emb @ W_ih.T + b, then 511-step LSTM
recurrence, returns (res[B,T-1,H], (hT[1,B,H], cT[1,B,H])).

Layout strategy (all on one NeuronCore; see notes below):
  - gates-on-partitions ("mapping A"): every on-chip tensor is [hidden-or-
    gate-slice=128 partitions, batch=32 free]. The recurrent matmul is
    out[g,b] += sum_h W_hh.T[h,g] * hT[h,b] with W tiles as the stationary
    operand (bf16 => fast weight load) and hT as the moving operand.
  - x_proj phase: emb rows are gathered with indirect DMA (token-major),
    PE-transposed to h-on-partitions, then multiplied against host-
    pre-transposed W_ih.T tiles (fp32), accumulating [gate=128, tok=512]
    tiles that are stored to DRAM in [m, t, p, b] order for the recurrence.
  - recurrence: 32 gate tiles x 8 k-chunks of bf16 matmuls into PSUM,
    epilogue (bias+xp add, sigmoid/tanh, c/h update) on DVE/ACT.

The per-step cross-core exchange needed for tensor-parallelism is not
viable in this runtime (collective_compute costs ~450us per op here), so
the recurrence runs on one core; W_hh/h are cast to bf16 for the matmul
(measured end-to-end rel-L2 error vs fp32 reference: ~4e-4).
"""

import numpy as np

B = 32
T = 512
T1 = T - 1          # 511 recurrence steps
TPAD = 512          # padded step count for the x_proj phase
H = 1024
G = 4 * H           # 4096 gate rows
V = 32000
KC = H // 128       # 8 contraction chunks
MT = G // 128       # 32 gate tiles
NTOK = TPAD * B     # 16384 padded tokens (order: (t, b), b fastest)
TGRP = 512          # tokens per x_proj matmul group
NGRP = NTOK // TGRP # 32 groups

_CACHE = {}


def _split_waits(nc, mybir, maxw=1):
    """This walrus build only accepts one sync-wait command per ctrl
    instruction; hoist excess waits onto preceding same-engine drains."""
    cnt = 0
    for f in nc.m.functions:
        for blk in f.blocks:
            newinsts = []
            for inst in blk.instructions:
                si = getattr(inst, "sync_info", None)
                if si is not None and si.on_wait is not None and len(si.on_wait) > maxw:
                    excess = list(si.on_wait[maxw:])
                    si.on_wait = list(si.on_wait[:maxw])
                    while excess:
                        batch, excess = excess[:maxw], excess[maxw:]
                        cnt += 1
                        newinsts.append(mybir.InstDrain(
                            name=f"I-ws-{cnt}",
                            engine=inst.engine,
                            ins=[], outs=[],
                            sync_info=mybir.SyncInfo(on_wait=batch, on_update=[]),
                        ))
                newinsts.append(inst)
            blk.instructions = newinsts
    return cnt


def _build(nsteps=T1):
    import concourse.bass as bass
    import concourse.mybir as mybir
    from concourse.tile import TileContext
    from concourse.masks import make_identity

    f32 = mybir.dt.float32
    bf16 = mybir.dt.bfloat16
    i32 = mybir.dt.int32

    nc = bass.Bass("TRN2", target_bir_lowering=False, debug=False, num_devices=1)

    idx_in = nc.dram_tensor("idx", [NTOK, 1], i32, kind="ExternalInput")
    embt = nc.dram_tensor("embt", [V, H], f32, kind="ExternalInput")
    wiht = nc.dram_tensor("wiht", [H, G], f32, kind="ExternalInput")       # W_ih.T
    whht = nc.dram_tensor("whht", [H, G], bf16, kind="ExternalInput")      # W_hh.T
    biasb = nc.dram_tensor("biasb", [128, MT], f32, kind="ExternalInput")  # [p, m]
    h0t = nc.dram_tensor("h0t", [128, KC * B], f32, kind="ExternalInput")  # chunked h0.T
    c0t = nc.dram_tensor("c0t", [128, KC * B], f32, kind="ExternalInput")
    res = nc.dram_tensor("res", [nsteps, 128, KC * B], f32, kind="ExternalOutput")
    ct = nc.dram_tensor("ct", [128, KC * B], f32, kind="ExternalOutput")

    xp_d = nc.dram_tensor("xp_scratch", [MT, TPAD, 128, B], f32, kind="Internal")

    with TileContext(nc) as tc:
        # ---------------- phase 1: gather + transpose + x_proj ----------
        with tc.tile_pool(name="wih_p", bufs=1) as wih_p, \
             tc.tile_pool(name="gath_p", bufs=3) as gath_p, \
             tc.tile_pool(name="embt_p", bufs=2) as embt_p, \
             tc.tile_pool(name="misc_p", bufs=1) as misc_p, \
             tc.tile_pool(name="tpsum_p", bufs=4, space="PSUM") as tpsum_p, \
             tc.tile_pool(name="xpsum_p", bufs=4, space="PSUM") as xpsum_p:

            ident = misc_p.tile([128, 128], f32, tag="ident")
            make_identity(nc, ident[:, :])

            idx_sb = misc_p.tile([128, NTOK // 128], i32, tag="idx")
            # token tile i, row p  <-  idx[i*128 + p]
            nc.sync.dma_start(
                idx_sb[:, :].rearrange("p (i o) -> p i o", o=1),
                idx_in.ap().rearrange("(i p) o -> p i o", p=128),
            )

            # W_ih.T resident in SBUF: [p, k, g] (k-chunk-major free dim)
            wih_sb = wih_p.tile([128, KC * G], f32, tag="wih")
            nc.sync.dma_start(
                wih_sb[:, :].rearrange("p (k g) -> p k g", k=KC),
                wiht.ap().rearrange("(k p) g -> p k g", p=128),
            )

            for grp in range(NGRP):
                # gather 4 token tiles (512 tokens) and transpose to
                # embT [p=h-chunk, k, tok-in-group]
                et = embt_p.tile([128, KC * TGRP], f32, tag="embt")
                for sub in range(TGRP // 128):
                    ti = grp * 4 + sub
                    gt = gath_p.tile([128, H], f32, tag="gath")
                    nc.gpsimd.indirect_dma_start(
                        out=gt[:, :],
                        out_offset=None,
                        in_=embt[:, :],
                        in_offset=bass.IndirectOffsetOnAxis(
                            ap=idx_sb[:, ti:ti + 1], axis=0),
                    )
                    for k in range(KC):
                        tp = tpsum_p.tile([128, 128], f32, tag="tps", space="PSUM")
                        nc.tensor.transpose(
                            tp[:, :], gt[:, k * 128:(k + 1) * 128], ident[:, :])
                        nc.scalar.copy(
                            et[:, k * TGRP + sub * 128: k * TGRP + (sub + 1) * 128],
                            tp[:, :])
                # x_proj for this token group: out tiles [g=128, tok=512]
                t0 = grp * (TGRP // B)  # first step index of this group
                for m in range(MT):
                    ps = xpsum_p.tile([128, TGRP], f32, tag="xps", space="PSUM")
                    for k in range(KC):
                        nc.tensor.matmul(
                            ps[:, :],
                            wih_sb[:, k * G + m * 128: k * G + (m + 1) * 128],
                            et[:, k * TGRP:(k + 1) * TGRP],
                            start=(k == 0), stop=(k == KC - 1),
                        )
                    # PSUM is not DMA-readable here: bounce via SBUF
                    xo = embt_p.tile([128, TGRP], f32, tag="xpout")
                    nc.vector.tensor_copy(xo[:, :], ps[:, :])
                    nc.sync.dma_start(
                        xp_d[m, t0:t0 + TGRP // B, :, :].rearrange(
                            "t p b -> p t b"),
                        xo[:, :].rearrange("p (t b) -> p t b", b=B),
                    )

        # ---------------- phase 2: recurrence ---------------------------
        with tc.tile_pool(name="whh_p", bufs=1) as whh_p, \
             tc.tile_pool(name="st_p", bufs=2) as st_p, \
             tc.tile_pool(name="xp_p", bufs=2) as xp_p, \
             tc.tile_pool(name="ep_p", bufs=3) as ep_p, \
             tc.tile_pool(name="gpsum_p", bufs=6, space="PSUM") as gpsum_p:

            whh_sb = whh_p.tile([128, KC * G], bf16, tag="whh")
            nc.sync.dma_start(
                whh_sb[:, :].rearrange("p (k g) -> p k g", k=KC),
                whht.ap().rearrange("(k p) g -> p k g", p=128),
            )
            bias_sb = whh_p.tile([128, MT], f32, tag="bias")
            nc.sync.dma_start(bias_sb[:, :], biasb[:, :])

            h_sb = st_p.tile([128, KC * B], f32, tag="h")
            nc.sync.dma_start(h_sb[:, :], h0t[:, :])
            c_sb = st_p.tile([128, KC * B], f32, tag="c")
            nc.sync.dma_start(c_sb[:, :], c0t[:, :])

            for t in range(nsteps):
                xp_sb = xp_p.tile([128, MT * B], f32, tag="xp")
                nc.sync.dma_start(
                    xp_sb[:, :].rearrange("p (m b) -> p m b", b=B),
                    xp_d[:, t, :, :].rearrange("m p b -> p m b"),
                )
                hbf = ep_p.tile([128, KC * B], bf16, tag="hbf")
                nc.vector.tensor_copy(hbf[:, :], h_sb[:, :])

                gin = ep_p.tile([128, MT * B], f32, tag="gin")
                for m in range(MT):
                    ps = gpsum_p.tile([128, B], f32, tag="gps", space="PSUM")
                    for k in range(KC):
                        nc.tensor.matmul(
                            ps[:, :],
                            whh_sb[:, k * G + m * 128: k * G + (m + 1) * 128],
                            hbf[:, k * B:(k + 1) * B],
                            start=(k == 0), stop=(k == KC - 1),
                        )
                    # gin = psum + bias[m] + xp[m]
                    nc.vector.scalar_tensor_tensor(
                        out=gin[:, m * B:(m + 1) * B],
                        in0=ps[:, :],
                        scalar=bias_sb[:, m:m + 1],
                        in1=xp_sb[:, m * B:(m + 1) * B],
                        op0=mybir.AluOpType.add,
                        op1=mybir.AluOpType.add,
                    )
                # gate tile order in gin free dim: m = gtype*8 + chunk
                Q = 8 * B  # 256 columns per gate type
                act = ep_p.tile([128, MT * B], f32, tag="act")
                nc.scalar.activation(      # i and f together
                    act[:, 0:2 * Q], gin[:, 0:2 * Q],
                    mybir.ActivationFunctionType.Sigmoid)
                nc.scalar.activation(      # g
                    act[:, 2 * Q:3 * Q], gin[:, 2 * Q:3 * Q],
                    mybir.ActivationFunctionType.Tanh)
                nc.scalar.activation(      # o
                    act[:, 3 * Q:4 * Q], gin[:, 3 * Q:4 * Q],
                    mybir.ActivationFunctionType.Sigmoid)

                ig = ep_p.tile([128, Q], f32, tag="ig")
                nc.vector.tensor_tensor(
                    ig[:, :], act[:, 0:Q], act[:, 2 * Q:3 * Q],
                    mybir.AluOpType.mult)
                fc = ep_p.tile([128, Q], f32, tag="fc")
                nc.vector.tensor_tensor(
                    fc[:, :], act[:, Q:2 * Q], c_sb[:, :], mybir.AluOpType.mult)
                c_sb = st_p.tile([128, KC * B], f32, tag="c")
                nc.vector.tensor_tensor(
                    c_sb[:, :], ig[:, :], fc[:, :], mybir.AluOpType.add)
                tanc = ep_p.tile([128, Q], f32, tag="tanc")
                nc.scalar.activation(
                    tanc[:, :], c_sb[:, :], mybir.ActivationFunctionType.Tanh)
                h_sb = st_p.tile([128, KC * B], f32, tag="h")
                nc.vector.tensor_tensor(
                    h_sb[:, :], act[:, 3 * Q:4 * Q], tanc[:, :],
                    mybir.AluOpType.mult)

                nc.sync.dma_start(res[t, :, :], h_sb[:, :])
            nc.sync.dma_start(ct[:, :], c_sb[:, :])

    _split_waits(nc, mybir)
    return nc


def _prep_inputs(tgt, h0, c0, emb_table, W_ih, W_hh, b_ih, b_hh, nsteps=T1):
    x = np.asarray(tgt)[:, :T1].astype(np.int32)          # [B, T1]
    idx = np.zeros((TPAD, B), np.int32)
    idx[:T1] = x.T                                        # (t, b) order
    bias = (np.asarray(b_ih) + np.asarray(b_hh)).astype(np.float32)

    import ml_dtypes
    h0f = np.asarray(h0, np.float32)[0]                   # [B, H]
    c0f = np.asarray(c0, np.float32)[0]
    # chunked transpose: [p, k*B] with col k*B+b = v[b, k*128+p]
    def chunkT(v):
        return np.ascontiguousarray(
            v.reshape(B, KC, 128).transpose(2, 1, 0).reshape(128, KC * B))
    return {
        "idx": idx.reshape(NTOK, 1),
        "embt": np.ascontiguousarray(np.asarray(emb_table, np.float32)),
        "wiht": np.ascontiguousarray(np.asarray(W_ih, np.float32).T),
        "whht": np.ascontiguousarray(
            np.asarray(W_hh, np.float32).T.astype(ml_dtypes.bfloat16)),
        "biasb": np.ascontiguousarray(bias.reshape(MT, 128).T),
        "h0t": chunkT(h0f),
        "c0t": chunkT(c0f),
    }


def _get_executable(nsteps):
    """Build + jit once per step count; returns (fn, in_names, out_names,
    zero_outs). Re-lowering the ~190MB BIR per call costs ~15s, so the
    jitted callable is cached and fed device arrays directly."""
    import jax
    import concourse.mybir as mybir
    from concourse import bass2jax

    key = ("exe", nsteps)
    if key in _CACHE:
        return _CACHE[key]
    nc = _build(nsteps)
    bass2jax.install_neuronx_cc_hook()
    pname = nc.partition_id_tensor.name if nc.partition_id_tensor else None
    in_names, out_names, out_avals, zeros = [], [], [], []
    for alloc in nc.m.functions[0].allocations:
        if not isinstance(alloc, mybir.MemoryLocationSet):
            continue
        name = alloc.memorylocations[0].name
        if alloc.kind == "ExternalInput":
            if name != pname:
                in_names.append(name)
        elif alloc.kind == "ExternalOutput":
            shape = tuple(alloc.tensor_shape)
            dt = mybir.dt.np(alloc.dtype)
            out_names.append(name)
            out_avals.append(jax.core.ShapedArray(shape, dt))
            zeros.append(np.zeros(shape, dt))
    all_names = in_names + out_names + ([pname] if pname else [])

    def _body(*args):
        ops = list(args)
        if pname:
            ops.append(bass2jax.partition_id_tensor())
        return tuple(bass2jax._bass_exec_p.bind(
            *ops, out_avals=tuple(out_avals), in_names=tuple(all_names),
            out_names=tuple(out_names), lowering_input_output_aliases=(),
            sim_require_finite=True, sim_require_nnan=True, nc=nc))

    fn = jax.jit(_body, keep_unused=True)
    _CACHE[key] = (fn, in_names, out_names, zeros)
    return _CACHE[key]


def kernel(tgt, h0, c0, encoder_outputs, src_lengths, emb_table, W_ih, W_hh,
           b_ih, b_hh, nsteps=T1):
    import jax

    fn, in_names, out_names, zeros = _get_executable(nsteps)
    in_map = _prep_inputs(tgt, h0, c0, emb_table, W_ih, W_hh, b_ih, b_hh,
                          nsteps=nsteps)
    dev = jax.devices()[0]
    args = [jax.device_put(in_map[n], dev) for n in in_names] +            [jax.device_put(z, dev) for z in zeros]
    outs = fn(*args)
    out = {name: np.asarray(outs[i]) for i, name in enumerate(out_names)}

    # res_dev [nsteps, 128, KC*B]: col k*B+b, row p -> h[b, k*128+p]
    r = out["res"].reshape(nsteps, 128, KC, B)
    res = np.ascontiguousarray(r.transpose(3, 0, 2, 1).reshape(B, nsteps, H))
    hT = res[:, -1, :][None]
    cT = np.ascontiguousarray(
        out["ct"].reshape(128, KC, B).transpose(2, 1, 0).reshape(B, H))[None]
    return res, (hT, cT)
